# revision 8
# baseline (speedup 1.0000x reference)
"""FullAttention (non-standard multiplicative causal mask) on 8 TRN2 cores.

Reference (per batch b, head h):
    S = Q @ K^T                      [L, L]
    S = S * tril(ones)               (multiplicative mask: zeros above diag)
    A = softmax(S / sqrt(E))         (masked slots contribute exp(0)=1)
    O = A @ V

Key ideas over the straightforward flash-style kernel:

1. PV layout swap: instead of streaming 512 l-columns with [V|1] stationary
   (cost = l-columns), PV streams only the 65 d-columns with P^T stationary
   (matmul cost is the moving/free size of the output; stationary loads are
   free).  PV column count per head drops 17408 -> 8840.

2. The exp over the causal area (the Act engine's 1 elem/cycle/lane is the
   other bottleneck) is SPLIT between the Scalar/Act engine (exact LUT exp)
   and the Vector/DVE engine running a custom 8-stage DVE op (EXP2BITS_ANT)
   that computes the bf16 BIT PATTERN of 2^w directly:
      r = 128*round(W/128) via the float magic-number trick (exact),
      s = W - r in [-64, 64),
      bits = (a2*s + a1)*s + a0 + r,  written with the f32->int16
      write-convert (RTNE, validated bit-exact on HW), aliased as bf16.
   The required input W = 128*log2(e)*SCALE*(q.k) + 16192 is produced by the
   QK matmul itself: q is pre-scaled host-side and a 65th contraction row
   (bias row q=126.5, k=128) adds the constant -- extra contraction rows are
   free (matmul cost is per-column).  Max rel err of this path ~0.6%, below
   the bf16 input noise floor.  The Act pieces use the same biased W via
   exp(W*ln2/128 - 126.5*ln2).

3. Pieces are assigned to Act vs DVE by a greedy build-time balance of
   modeled busy-time; diagonal pieces (sub-slice APs, 2 free dims) must go
   to Act because the custom op needs a flat 1-free-dim AP (full-tensor
   Src1 + imm2 constraint; [P,1]-broadcast Src1 crashes this firmware).

Sharding: B*H = 32 (b,h) pairs -> 4 per core (2 "packs" of 2 heads).
Per (b,h), chunk-outer loop over l-chunks of 512, s_tile pieces of 128.
The PV output accumulates [l_tile 128, 65] per (head, l_tile) in PSUM
(num cols 0:64, denominator col 64 via the ones column of V1); the host
adds the exact f32 suffix sums (s_tiles > l's tile), divides, reshapes.
"""

import numpy as np

import concourse.bass as bass
import concourse.mybir as mybir
import concourse.tile as tile
from concourse import bacc

F32 = mybir.dt.float32
BF16 = mybir.dt.bfloat16
I16 = mybir.dt.int16
AF = mybir.ActivationFunctionType

B, L, H, E = 2, 2048, 16, 64
D = 64
SCALE = 0.125          # 1/sqrt(64)
NCORES = 8
BH_PER_CORE = (B * H) // NCORES   # 4
PACKS = BH_PER_CORE // 2          # 2
NT = L // 128                     # 16 s-tiles
NJ = L // 512                     # 4 l-chunks
VW = 66                           # [V | 1 | 0pad] (cols 0:65 used)

LN2 = float(np.log(2.0))
QS = 128.0 * SCALE / LN2          # 128*SCALE*log2(e) = 23.0831...
BIASQ = 126.5                     # bias row values: W += 126.5*128 = 16192
BIASK = 128.0
MAGIC = float(np.float32(1.5 * 2 ** 30))
# minimax fit of 128*(2^((s+64)/128)-1) on s in [-64, 64), rel-weighted
E2A2, E2A1, E2A0 = 0.0026273895836556836, 0.9930504397343359, 53.08995260214826

# cost model (ns) for greedy Act/DVE balance
ACT_NS = 1.0 / 1.2
DVE_NS = 1.0 / 0.96
ACT_INIT = 185.0
DVE_INIT = 125.0

_cached = None


def _register_exp2bits():
    from concourse import dve_ops
    from concourse.dve_spec import Spec, Src0, Src1, C0, C1, C2, lower, _has_src1
    from concourse.dve_uop import DveOpSpec

    name = "EXP2BITS_ANT"
    for op in dve_ops.OPS:
        if op.name == name:
            return op
    m = Src0 + C0
    r = m - C0
    s = Src0 - r
    body = ((s * C1 + C2) * s + Src1) + r

    def ref(in0, in1, s0, s1, imm2):
        in0 = in0.astype(np.float32)
        mm = (in0 + np.float32(s0)).astype(np.float32)
        rr = (mm - np.float32(s0)).astype(np.float32)
        ss = (in0 - rr).astype(np.float32)
        a0 = np.asarray(in1, np.float32)
        if a0.ndim:
            a0 = a0.reshape(in0.shape[0], -1)[:, : in0.shape[-1] if in0.ndim == 2 else 1]
        return ((ss * np.float32(s1) + np.float32(imm2)) * ss + a0 + rr).astype(
            np.float32
        )

    spec = Spec(body=body, reference=ref)
    row = dve_ops._CUSTOM_DVE_ROW_BASE + len(dve_ops.OPS)
    assert row < 0x20
    dve_ops._SUB_OPCODE_FOR_NAME[name] = row
    sha = DveOpSpec(
        name=name, opcode=row, uops=lower(spec, ver="v3"), rd1_en=_has_src1(spec)
    ).sha("v3")
    op = dve_ops.DveOp(name, spec, subdim=False, uops_sha={"v3": sha})
    dve_ops.OPS.append(op)
    dve_ops.CUSTOM_DVE_SPECS[name] = op.spec
    return op


def _build_program():
    EXP2 = _register_exp2bits()
    nc = bacc.Bacc("TRN2", target_bir_lowering=False)
    qt = nc.dram_tensor("qt", [PACKS, 2, 65, L], BF16, kind="ExternalInput")
    kt = nc.dram_tensor("kt", [PACKS, 2, 65, L], BF16, kind="ExternalInput")
    v1d = nc.dram_tensor("v1", [BH_PER_CORE, 128, NT * VW], BF16,
                         kind="ExternalInput")
    ob = nc.dram_tensor("ob", [BH_PER_CORE, NJ, 2, 128, 130], F32,
                        kind="ExternalOutput")

    with tile.TileContext(nc) as tc:
        with (
            tc.tile_pool(name="consts", bufs=1) as consts,
            tc.tile_pool(name="qk_sb", bufs=2) as qk_sb,
            tc.tile_pool(name="v1_sb", bufs=4) as v1_pool,
            tc.tile_pool(name="pt", bufs=8) as pt_pool,
            tc.tile_pool(name="ot_sb", bufs=4) as ot_sb_pool,
            tc.tile_pool(name="qkps", bufs=2, space="PSUM") as qk_ps,
            tc.tile_pool(name="pvps", bufs=4, space="PSUM") as pv_ps,
        ):
            # constants: a0 tile for the custom op (full tensor: [P,1]
            # broadcast Src1 crashes this firmware), Act bias, PE warm tile
            a0full = consts.tile([128, 1024], F32)
            nc.gpsimd.memset(a0full, E2A0)
            biast = consts.tile([128, 1], F32)
            nc.vector.memset(biast, -BIASQ * LN2)
            warm_sb = consts.tile([128, 64], BF16)
            nc.vector.memset(warm_sb, 0.25)
            # warm the PE p-state during the input-DMA window (ramp needs
            # busy-time, not columns) and pre-load the Act exp table
            warm_ps = qk_ps.tile([128, 2, 512], F32, tag="pp", name="warm")
            for w in range(6):
                nc.tensor.matmul(
                    warm_ps[0:64, 0, 0:64], warm_sb[:, 0:64], warm_sb,
                    start=True, stop=True, skip_group_check=True,
                )
            warm_act = consts.tile([128, 16], BF16)
            nc.scalar.activation(warm_act[:, :], warm_sb[:, 0:16], AF.Exp,
                                 scale=LN2 / 128.0, bias=biast[:, :])
            # zero operands for the PV-bank clearing matmul (one K=1 matmul
            # zeroes a whole [128, 260] accumulator region; per-group
            # start=True matmuls corrupt sibling groups in the same bank)
            zlhs = consts.tile([1, 128], BF16)
            nc.vector.memset(zlhs, 0.0)
            zrhs = consts.tile([1, 260], BF16)
            nc.vector.memset(zrhs, 0.0)

            pack_tiles = {}
            busy = {"A": 0.0, "D": 0.0}

            def pick_engine(cols, force_act=False):
                ca = cols * ACT_NS + ACT_INIT
                cd = cols * DVE_NS + DVE_INIT
                if force_act or busy["A"] + ca <= busy["D"] + cd:
                    busy["A"] += ca
                    return "A"
                busy["D"] += cd
                return "D"

            def load_pack(p):
                # chunk-sliced loads: QK of chunk j only needs q cols
                # [512j, 512j+512) and k cols [0, 512(j+1))
                qt_t = qk_sb.tile([65, 2, NJ, 512], BF16, tag="qt", name="qt_t")
                kt_t = qk_sb.tile([65, 2, NJ, 512], BF16, tag="kt", name="kt_t")
                v1l = []
                for hh2 in range(2):
                    v1l.append(v1_pool.tile([128, NJ, 4 * VW], BF16, tag="v1",
                                            name="v1_t"))
                for j in range(NJ):
                    for hh2 in range(2):
                        # k on sync, q on gpsimd: first slices land in
                        # parallel; piece 0 only needs k cols [0:128)
                        if j == 0:
                            nc.sync.dma_start(out=kt_t[:, hh2, 0, 0:128],
                                              in_=kt[p, hh2, :, 0:128])
                            nc.sync.dma_start(out=kt_t[:, hh2, 0, 128:512],
                                              in_=kt[p, hh2, :, 128:512])
                        else:
                            nc.sync.dma_start(
                                out=kt_t[:, hh2, j, :],
                                in_=kt[p, hh2, :, 512 * j:512 * (j + 1)])
                        nc.gpsimd.dma_start(
                            out=qt_t[:, hh2, j, :],
                            in_=qt[p, hh2, :, 512 * j:512 * (j + 1)])
                        nc.scalar.dma_start(
                            out=v1l[hh2][:, j, :],
                            in_=v1d[2 * p + hh2, :,
                                    4 * VW * j:4 * VW * (j + 1)],
                        )
                pack_tiles[p] = (qt_t, kt_t, v1l)

            load_pack(0)
            for pack in range(PACKS):
                qt_t, kt_t, v1 = pack_tiles.pop(pack)

                j_iter = range(NJ) if pack == 0 else range(NJ - 1, -1, -1)
                for j in j_iter:
                    if pack == 0 and j == 2 and pack + 1 < PACKS:
                        load_pack(pack + 1)
                    nk = 4 * j + 4          # s_tiles participating causally
                    # PV accumulators: 2 PSUM tiles per chunk, one per
                    # l_tile pair (hh, li&1, col); zeroed by a single K=1
                    # matmul, then all PV matmuls accumulate (start=False)
                    pv = [pv_ps.tile([128, 2, 2, 65], F32, tag="pv",
                                     name="pv") for _ in range(2)]
                    for lp in range(2):
                        nc.tensor.matmul(
                            pv[lp].rearrange("p a b c -> p (a b c)")[:, :],
                            zlhs[:, :], zrhs[:, :],
                            start=True, stop=False, skip_group_check=True,
                        )

                    pending_pv = []   # list of per-piece lists of closures

                    def emit_pv(hh, k, li, pt_t, j=j, pv=pv, v1=v1):
                        nc.tensor.matmul(
                            pv[li // 2][:, hh, li % 2, :],
                            pt_t[:, hh, 128 * li:128 * li + 128],
                            v1[hh][:, k // 4,
                                   VW * (k % 4):VW * (k % 4) + 65],
                            start=False,
                            stop=(k == 4 * j + li),
                            skip_group_check=True,
                        )

                    def exp_piece(eng, pt_ap, pp_ap, ncols):
                        if eng == "A":
                            nc.scalar.activation(
                                pt_ap, pp_ap,
                                AF.Exp, scale=LN2 / 128.0, bias=biast[:, :],
                            )
                        else:
                            nc.vector._custom_dve(
                                EXP2,
                                out=pt_ap.bitcast(I16),
                                in0=pp_ap,
                                in1=a0full[:, 0:ncols],
                                s0=MAGIC,
                                s1=E2A2,
                                imm2=E2A1,
                            )

                    for k in range(nk):             # s_tile pieces
                        # drain delayed PV BEFORE emitting the next QK: the
                        # PE is in-order, so ready PV work must sit ahead of
                        # a QK that may stall on its PSUM bank
                        depth = 2 if (pack == PACKS - 1 and j == 0) else 6
                        while len(pending_pv) > depth:
                            for fn in pending_pv.pop(0):
                                fn()
                        pp = qk_ps.tile([128, 2, 512], F32, tag="pp",
                                        name="pp")
                        pt_t = pt_pool.tile([128, 2, 512], BF16, tag="pt",
                                            name="pt")
                        m = k - 4 * j
                        qoff = 128 * max(0, m)
                        for hh in range(2):
                            nc.tensor.matmul(
                                pp[:, hh, qoff:512],
                                kt_t[:, hh, k // 4,
                                     128 * (k % 4):128 * (k % 4) + 128],
                                qt_t[:, hh, j, qoff:512],
                                start=True, stop=True,
                            )
                        # exp: diagonal pieces split per-head across BOTH
                        # engines (halves the PSUM-bank release latency and
                        # avoids consecutive same-engine runs); full pieces
                        # merged 2-head on the greedy-min engine (greedy
                        # naturally alternates when balanced)
                        cols1 = 512 - qoff
                        if m >= 1:
                            e0 = pick_engine(cols1)
                            e1 = "D" if e0 == "A" else "A"
                            busy[e1] += (cols1 * (ACT_NS if e1 == "A"
                                                  else DVE_NS)
                                         + (ACT_INIT if e1 == "A"
                                            else DVE_INIT))
                            for hh, eng in ((0, e0), (1, e1)):
                                exp_piece(eng,
                                          pt_t[:, hh, qoff:512],
                                          pp[:, hh, qoff:512], cols1)
                        else:
                            eng = pick_engine(2 * cols1)
                            if eng == "A":
                                exp_piece("A", pt_t[:, :, :], pp[:, :, :],
                                          1024)
                            else:
                                exp_piece(
                                    "D",
                                    pt_t.rearrange("p a b -> p (a b)")[:, :],
                                    pp.rearrange("p a b -> p (a b)")[:, :],
                                    1024)
                        piece_pv = []
                        for hh in range(2):
                            if m >= 0:             # diagonal piece fixup
                                nc.gpsimd.affine_select(
                                    out=pt_t[:, hh, 128 * m:128 * m + 128],
                                    in_=pt_t[:, hh, 128 * m:128 * m + 128],
                                    compare_op=mybir.AluOpType.is_ge,
                                    fill=1.0,
                                    base=0,
                                    pattern=[[1, 128]],
                                    channel_multiplier=-1,
                                )
                            for li in range(max(0, m), 4):
                                piece_pv.append(
                                    lambda hh=hh, k=k, li=li, p=pt_t:
                                    emit_pv(hh, k, li, p)
                                )
                        pending_pv.append(piece_pv)
                    for plist in pending_pv:
                        for fn in plist:
                            fn()
                    pending_pv = []

                    # ship [128, 2, 2, 65] f32 per l_tile-pair; suffix add +
                    # divide happen on the host
                    for lp in range(2):
                        ot_s = ot_sb_pool.tile([128, 2, 2, 65], F32,
                                               tag="ots")
                        ccols = 260.0
                        ca = ccols * ACT_NS + ACT_INIT
                        cd = ccols * DVE_NS + DVE_INIT
                        if busy["A"] + ca <= busy["D"] + cd:
                            busy["A"] += ca
                            nc.scalar.copy(ot_s[:, :, :, :], pv[lp][:, :, :, :])
                        else:
                            busy["D"] += cd
                            nc.vector.tensor_copy(ot_s[:, :, :, :], pv[lp][:, :, :, :])
                        for hh in range(2):
                            bh = 2 * pack + hh
                            nc.gpsimd.dma_start(
                                out=ob[bh, j, lp],
                                in_=ot_s.rearrange("p a b c -> p a (b c)")[
                                    :, hh, :],
                            )

    nc.compile()
    return nc


def _get_program():
    global _cached
    if _cached is None:
        _cached = _build_program()
    return _cached


def _shard_inputs(queries, keys, values):
    import ml_dtypes
    BF = ml_dtypes.bfloat16
    # [B, L, H, E] -> [B, H, E, L] -> [BH, E, L]
    qT = np.ascontiguousarray(queries.transpose(0, 2, 3, 1)).reshape(B * H, E, L)
    kT = np.ascontiguousarray(keys.transpose(0, 2, 3, 1)).reshape(B * H, E, L)
    # [B, L, H, D] -> [BH, L, D]
    vv = np.ascontiguousarray(values.transpose(0, 2, 1, 3)).reshape(B * H, L, D)
    in_maps = []
    sufs = []
    for c in range(NCORES):
        s = c * BH_PER_CORE
        qa = np.zeros((PACKS, 2, 65, L), dtype=np.float32)
        ka = np.zeros((PACKS, 2, 65, L), dtype=np.float32)
        for p in range(PACKS):
            for hh in range(2):
                bh = s + 2 * p + hh
                qa[p, hh, 0:64] = qT[bh] * QS
                qa[p, hh, 64] = BIASQ
                ka[p, hh, 0:64] = kT[bh]
                ka[p, hh, 64] = BIASK
        vb = vv[s:s + BH_PER_CORE].reshape(BH_PER_CORE, NT, 128, D)
        v1h = np.zeros((BH_PER_CORE, 128, NT, VW), dtype=np.float32)
        v1h[:, :, :, 0:64] = vb.transpose(0, 2, 1, 3)
        v1h[:, :, :, 64] = 1.0
        # suffix tables (f32, exact V), applied host-side at gather:
        # SUF[t] = sum over s_tiles > t of [V|1|0] rows (col 64 = count)
        vrows = v1h.transpose(0, 2, 1, 3).sum(axis=2)  # [BH, NT, VW]
        suf = np.zeros((BH_PER_CORE, NT, VW), dtype=np.float32)
        suf[:, :-1] = vrows[:, ::-1].cumsum(axis=1)[:, -2::-1]
        sufs.append(suf)
        in_maps.append({
            "qt": qa.astype(BF),
            "kt": ka.astype(BF),
            "v1": v1h.astype(BF).reshape(BH_PER_CORE, 128, NT * VW),
        })
    return in_maps, sufs


def _gather_outputs(results, sufs):
    outs = []
    for r, suf in zip(results, sufs):
        acc = np.asarray(r["ob"], dtype=np.float32)   # [BH, NJ, 2, 128, 130]
        acc = acc.reshape(BH_PER_CORE, NJ, 2, 128, 2, 65)
        # l_tile t = 4j + 2*lp + i
        acc = acc.transpose(0, 1, 2, 4, 3, 5).reshape(BH_PER_CORE, NT, 128, 65)
        num = acc[..., 0:64] + suf[:, :, None, 0:64]
        den = acc[..., 64] + suf[:, :, None, 64]
        o = num / den[..., None]                      # [BH, NT, 128, 64]
        outs.append(o.reshape(BH_PER_CORE, L, D))
    full = np.concatenate(outs, axis=0)               # [B*H, L, D]
    return np.ascontiguousarray(
        full.reshape(B, H, L, D).transpose(0, 2, 1, 3)
    ).astype(np.float32)  # [B, L, H, D]


def kernel(queries, keys, values, _trace=[False]):
    from concourse.bass_utils import run_bass_kernel_spmd

    queries = np.asarray(queries, dtype=np.float32)
    keys = np.asarray(keys, dtype=np.float32)
    values = np.asarray(values, dtype=np.float32)
    nc = _get_program()
    in_maps, sufs = _shard_inputs(queries, keys, values)
    res = run_bass_kernel_spmd(
        nc, in_maps, core_ids=list(range(NCORES)), trace=_trace[0]
    )
    out = _gather_outputs(res.results, sufs)
    if _trace[0]:
        kernel.last_results = res
    return out


# revision 9
# speedup vs baseline: 1.2203x; 1.2203x over previous
"""FullAttention (non-standard multiplicative causal mask) on 8 TRN2 cores.

Reference (per batch b, head h):
    S = Q @ K^T                      [L, L]
    S = S * tril(ones)               (multiplicative mask: zeros above diag)
    A = softmax(S / sqrt(E))         (masked slots contribute exp(0)=1)
    O = A @ V

Key ideas over the straightforward flash-style kernel:

1. PV layout swap: instead of streaming 512 l-columns with [V|1] stationary
   (cost = l-columns), PV streams only the 65 d-columns with P^T stationary
   (matmul cost is the moving/free size of the output; stationary loads are
   free).  PV column count per head drops 17408 -> 8840.

2. The exp over the causal area (the Act engine's 1 elem/cycle/lane is the
   other bottleneck) is SPLIT between the Scalar/Act engine (exact LUT exp)
   and the Vector/DVE engine running a custom 8-stage DVE op (EXP2BITS_ANT)
   that computes the bf16 BIT PATTERN of 2^w directly:
      r = 128*round(W/128) via the float magic-number trick (exact),
      s = W - r in [-64, 64),
      bits = (a2*s + a1)*s + a0 + r,  written with the f32->int16
      write-convert (RTNE, validated bit-exact on HW), aliased as bf16.
   The required input W = 128*log2(e)*SCALE*(q.k) + 16192 is produced by the
   QK matmul itself: q is pre-scaled host-side and a 65th contraction row
   (bias row q=126.5, k=128) adds the constant -- extra contraction rows are
   free (matmul cost is per-column).  Max rel err of this path ~0.6%, below
   the bf16 input noise floor.  The Act pieces use the same biased W via
   exp(W*ln2/128 - 126.5*ln2).

3. Pieces are assigned to Act vs DVE by a greedy build-time balance of
   modeled busy-time; diagonal pieces (sub-slice APs, 2 free dims) must go
   to Act because the custom op needs a flat 1-free-dim AP (full-tensor
   Src1 + imm2 constraint; [P,1]-broadcast Src1 crashes this firmware).

Sharding: B*H = 32 (b,h) pairs -> 4 per core (2 "packs" of 2 heads).
Per (b,h), chunk-outer loop over l-chunks of 512, s_tile pieces of 128.
The PV output accumulates [l_tile 128, 65] per (head, l_tile) in PSUM
(num cols 0:64, denominator col 64 via the ones column of V1); the host
adds the exact f32 suffix sums (s_tiles > l's tile), divides, reshapes.
"""

import numpy as np

import concourse.bass as bass
import concourse.mybir as mybir
import concourse.tile as tile
from concourse import bacc

F32 = mybir.dt.float32
BF16 = mybir.dt.bfloat16
I16 = mybir.dt.int16
AF = mybir.ActivationFunctionType

B, L, H, E = 2, 2048, 16, 64
D = 64
SCALE = 0.125          # 1/sqrt(64)
NCORES = 8
BH_PER_CORE = (B * H) // NCORES   # 4
PACKS = BH_PER_CORE // 2          # 2
NT = L // 128                     # 16 s-tiles
NJ = L // 512                     # 4 l-chunks
VW = 66                           # [V | 1 | 0pad] (cols 0:65 used)

LN2 = float(np.log(2.0))
QS = 128.0 * SCALE / LN2          # 128*SCALE*log2(e) = 23.0831...
MAGIC = float(np.float32(1.5 * 2 ** 30))
# minimax fit of the (kinked) round-variant bits function on s in [-64, 64):
# bits = p(s) + 128*round(w), s = W - 128*round(w), W = 128*w unbiased
E2A2, E2A1, E2A0 = -0.0024742558182972215, 1.0072715927101399, 16252.395694060908

# cost model (ns) for greedy Act/DVE balance
ACT_NS = 1.0 / 1.2
DVE_NS = 1.0 / 0.96
ACT_INIT = 185.0
DVE_INIT = 125.0

_cached = None


def _register_exp2bits():
    from concourse import dve_ops
    from concourse.dve_spec import Spec, Src0, Src1, C0, C1, C2, lower, _has_src1
    from concourse.dve_uop import DveOpSpec

    name = "EXP2BITS_ANT"
    for op in dve_ops.OPS:
        if op.name == name:
            return op
    m = Src0 + C0
    r = m - C0
    s = Src0 - r
    body = ((s * C1 + C2) * s + Src1) + r

    def ref(in0, in1, s0, s1, imm2):
        in0 = in0.astype(np.float32)
        mm = (in0 + np.float32(s0)).astype(np.float32)
        rr = (mm - np.float32(s0)).astype(np.float32)
        ss = (in0 - rr).astype(np.float32)
        a0 = np.asarray(in1, np.float32)
        if a0.ndim:
            a0 = a0.reshape(in0.shape[0], -1)[:, : in0.shape[-1] if in0.ndim == 2 else 1]
        return ((ss * np.float32(s1) + np.float32(imm2)) * ss + a0 + rr).astype(
            np.float32
        )

    spec = Spec(body=body, reference=ref)
    row = dve_ops._CUSTOM_DVE_ROW_BASE + len(dve_ops.OPS)
    assert row < 0x20
    dve_ops._SUB_OPCODE_FOR_NAME[name] = row
    sha = DveOpSpec(
        name=name, opcode=row, uops=lower(spec, ver="v3"), rd1_en=_has_src1(spec)
    ).sha("v3")
    op = dve_ops.DveOp(name, spec, subdim=False, uops_sha={"v3": sha})
    dve_ops.OPS.append(op)
    dve_ops.CUSTOM_DVE_SPECS[name] = op.spec
    return op


def _build_program():
    EXP2 = _register_exp2bits()
    nc = bacc.Bacc("TRN2", target_bir_lowering=False)
    qt = nc.dram_tensor("qt", [PACKS, 128, L], BF16, kind="ExternalInput")
    kt = nc.dram_tensor("kt", [PACKS, 128, L], BF16, kind="ExternalInput")
    v1d = nc.dram_tensor("v1", [BH_PER_CORE, 128, NT * VW], BF16,
                         kind="ExternalInput")
    ob = nc.dram_tensor("ob", [BH_PER_CORE, NJ, 2, 128, 130], F32,
                        kind="ExternalOutput")

    with tile.TileContext(nc) as tc:
        with (
            tc.tile_pool(name="consts", bufs=1) as consts,
            tc.tile_pool(name="qk_sb", bufs=2) as qk_sb,
            tc.tile_pool(name="v1_sb", bufs=4) as v1_pool,
            tc.tile_pool(name="pt", bufs=8) as pt_pool,
            tc.tile_pool(name="ot_sb", bufs=4) as ot_sb_pool,
            tc.tile_pool(name="qkps", bufs=2, space="PSUM") as qk_ps,
            tc.tile_pool(name="pvps", bufs=4, space="PSUM") as pv_ps,
        ):
            # constants: a0 tile for the custom op (full tensor: [P,1]
            # broadcast Src1 crashes this firmware), Act bias, PE warm tile
            a0full = consts.tile([128, 1024], F32)
            nc.gpsimd.memset(a0full, E2A0)
            warm_sb = consts.tile([128, 64], BF16)
            nc.vector.memset(warm_sb, 0.25)
            # warm the PE p-state during the input-DMA window (ramp needs
            # busy-time, not columns) and pre-load the Act exp table
            warm_ps = qk_ps.tile([128, 2, 512], F32, tag="pp", name="warm")
            for w in range(6):
                nc.tensor.matmul(
                    warm_ps[0:64, 0, 0:64], warm_sb[:, 0:64], warm_sb,
                    start=True, stop=True, skip_group_check=True,
                )
            warm_act = consts.tile([128, 16], BF16)
            nc.scalar.activation(warm_act[:, :], warm_sb[:, 0:16], AF.Exp,
                                 scale=LN2 / 128.0)
            # zero operands for the PV-bank clearing matmul (one K=1 matmul
            # zeroes a whole [128, 260] accumulator region; per-group
            # start=True matmuls corrupt sibling groups in the same bank)
            zlhs = consts.tile([1, 128], BF16)
            nc.vector.memset(zlhs, 0.0)
            zrhs = consts.tile([1, 260], BF16)
            nc.vector.memset(zrhs, 0.0)

            pack_tiles = {}
            busy = {"A": 0.0, "D": 0.0}

            def pick_engine(cols, force_act=False):
                ca = cols * ACT_NS + ACT_INIT
                cd = cols * DVE_NS + DVE_INIT
                if force_act or busy["A"] + ca <= busy["D"] + cd:
                    busy["A"] += ca
                    return "A"
                busy["D"] += cd
                return "D"

            def load_pack(p):
                # chunk-sliced loads: QK of chunk j only needs q cols
                # [512j, 512j+512) and k cols [0, 512(j+1))
                qt_t = qk_sb.tile([128, NJ, 512], BF16, tag="qt", name="qt_t")
                kt_t = qk_sb.tile([128, NJ, 512], BF16, tag="kt", name="kt_t")
                v1l = []
                for hh2 in range(2):
                    v1l.append(v1_pool.tile([128, NJ, 4 * VW], BF16, tag="v1",
                                            name="v1_t"))
                for j in range(NJ):
                    # k on sync, q on gpsimd: first slices land in parallel;
                    # piece 0 only needs k cols [0:128)
                    if j == 0:
                        nc.sync.dma_start(out=kt_t[:, 0, 0:128],
                                          in_=kt[p, :, 0:128])
                        nc.sync.dma_start(out=kt_t[:, 0, 128:512],
                                          in_=kt[p, :, 128:512])
                    else:
                        nc.sync.dma_start(out=kt_t[:, j, :],
                                          in_=kt[p, :, 512 * j:512 * (j + 1)])
                    nc.gpsimd.dma_start(out=qt_t[:, j, :],
                                        in_=qt[p, :, 512 * j:512 * (j + 1)])
                    for hh2 in range(2):
                        nc.sync.dma_start(
                            out=v1l[hh2][:, j, :],
                            in_=v1d[2 * p + hh2, :,
                                    4 * VW * j:4 * VW * (j + 1)],
                        )
                pack_tiles[p] = (qt_t, kt_t, v1l)

            load_pack(0)
            for pack in range(PACKS):
                qt_t, kt_t, v1 = pack_tiles.pop(pack)

                j_iter = range(NJ) if pack == 0 else range(NJ - 1, -1, -1)
                for j in j_iter:
                    if pack == 0 and j == 2 and pack + 1 < PACKS:
                        load_pack(pack + 1)
                    nk = 4 * j + 4          # s_tiles participating causally
                    # PV accumulators: 2 PSUM tiles per chunk, one per
                    # l_tile pair (hh, li&1, col); zeroed by a single K=1
                    # matmul, then all PV matmuls accumulate (start=False)
                    pv = [pv_ps.tile([128, 2, 2, 65], F32, tag="pv",
                                     name="pv") for _ in range(2)]
                    for lp in range(2):
                        nc.tensor.matmul(
                            pv[lp].rearrange("p a b c -> p (a b c)")[:, :],
                            zlhs[:, :], zrhs[:, :],
                            start=True, stop=False, skip_group_check=True,
                        )

                    pending_pv = []   # list of per-piece lists of closures

                    def emit_pv(hh, k, li, pt_t, j=j, pv=pv, v1=v1):
                        nc.tensor.matmul(
                            pv[li // 2][:, hh, li % 2, :],
                            pt_t[:, hh, 128 * li:128 * li + 128],
                            v1[hh][:, k // 4,
                                   VW * (k % 4):VW * (k % 4) + 65],
                            start=False,
                            stop=(k == 4 * j + li),
                            skip_group_check=True,
                        )

                    def exp_piece(eng, pt_ap, pp_ap, ncols):
                        if eng == "A":
                            nc.scalar.activation(
                                pt_ap, pp_ap,
                                AF.Exp, scale=LN2 / 128.0,
                            )
                        else:
                            nc.vector._custom_dve(
                                EXP2,
                                out=pt_ap.bitcast(I16),
                                in0=pp_ap,
                                in1=a0full[:, 0:ncols],
                                s0=MAGIC,
                                s1=E2A2,
                                imm2=E2A1,
                            )

                    for k in range(nk):             # s_tile pieces
                        # drain delayed PV BEFORE emitting the next QK: the
                        # PE is in-order, so ready PV work must sit ahead of
                        # a QK that may stall on its PSUM bank
                        depth = 2 if (pack == PACKS - 1 and j == 0) else 6
                        while len(pending_pv) > depth:
                            for fn in pending_pv.pop(0):
                                fn()
                        pp = qk_ps.tile([128, 2, 512], F32, tag="pp",
                                        name="pp")
                        pt_t = pt_pool.tile([128, 2, 512], BF16, tag="pt",
                                            name="pt")
                        m = k - 4 * j
                        qoff = 128 * max(0, m)
                        for hh in range(2):
                            r0 = 64 * hh
                            nc.tensor.matmul(
                                pp[:, hh, qoff:512],
                                kt_t[r0:r0 + 64, k // 4,
                                     128 * (k % 4):128 * (k % 4) + 128],
                                qt_t[r0:r0 + 64, j, lo + qoff - lo:512]
                                if False else
                                qt_t[r0:r0 + 64, j, qoff:512],
                                start=True, stop=True,
                            )
                        # exp: diagonal pieces split per-head across BOTH
                        # engines (halves the PSUM-bank release latency and
                        # avoids consecutive same-engine runs); full pieces
                        # merged 2-head on the greedy-min engine (greedy
                        # naturally alternates when balanced)
                        cols1 = 512 - qoff
                        if m >= 1:
                            e0 = pick_engine(cols1)
                            e1 = "D" if e0 == "A" else "A"
                            busy[e1] += (cols1 * (ACT_NS if e1 == "A"
                                                  else DVE_NS)
                                         + (ACT_INIT if e1 == "A"
                                            else DVE_INIT))
                            for hh, eng in ((0, e0), (1, e1)):
                                exp_piece(eng,
                                          pt_t[:, hh, qoff:512],
                                          pp[:, hh, qoff:512], cols1)
                        else:
                            eng = pick_engine(2 * cols1)
                            if eng == "A":
                                exp_piece("A", pt_t[:, :, :], pp[:, :, :],
                                          1024)
                            else:
                                exp_piece(
                                    "D",
                                    pt_t.rearrange("p a b -> p (a b)")[:, :],
                                    pp.rearrange("p a b -> p (a b)")[:, :],
                                    1024)
                        piece_pv = []
                        for hh in range(2):
                            if m >= 0:             # diagonal piece fixup
                                nc.gpsimd.affine_select(
                                    out=pt_t[:, hh, 128 * m:128 * m + 128],
                                    in_=pt_t[:, hh, 128 * m:128 * m + 128],
                                    compare_op=mybir.AluOpType.is_ge,
                                    fill=1.0,
                                    base=0,
                                    pattern=[[1, 128]],
                                    channel_multiplier=-1,
                                )
                            for li in range(max(0, m), 4):
                                piece_pv.append(
                                    lambda hh=hh, k=k, li=li, p=pt_t:
                                    emit_pv(hh, k, li, p)
                                )
                        pending_pv.append(piece_pv)
                    for plist in pending_pv:
                        for fn in plist:
                            fn()
                    pending_pv = []

                    # ship [128, 2, 2, 65] f32 per l_tile-pair; suffix add +
                    # divide happen on the host
                    for lp in range(2):
                        ot_s = ot_sb_pool.tile([128, 2, 2, 65], F32,
                                               tag="ots")
                        ccols = 260.0
                        ca = ccols * ACT_NS + ACT_INIT
                        cd = ccols * DVE_NS + DVE_INIT
                        if busy["A"] + ca <= busy["D"] + cd:
                            busy["A"] += ca
                            nc.scalar.copy(ot_s[:, :, :, :], pv[lp][:, :, :, :])
                        else:
                            busy["D"] += cd
                            nc.vector.tensor_copy(ot_s[:, :, :, :], pv[lp][:, :, :, :])
                        for hh in range(2):
                            bh = 2 * pack + hh
                            nc.sync.dma_start(
                                out=ob[bh, j, lp],
                                in_=ot_s.rearrange("p a b c -> p a (b c)")[
                                    :, hh, :],
                            )

    nc.compile()
    return nc


def _get_program():
    global _cached
    if _cached is None:
        _cached = _build_program()
    return _cached


def _shard_inputs(queries, keys, values):
    import ml_dtypes
    BF = ml_dtypes.bfloat16
    # [B, L, H, E] -> [B, H, E, L] -> [BH, E, L]
    qT = np.ascontiguousarray(queries.transpose(0, 2, 3, 1)).reshape(B * H, E, L)
    kT = np.ascontiguousarray(keys.transpose(0, 2, 3, 1)).reshape(B * H, E, L)
    # [B, L, H, D] -> [BH, L, D]
    vv = np.ascontiguousarray(values.transpose(0, 2, 1, 3)).reshape(B * H, L, D)
    in_maps = []
    sufs = []
    for c in range(NCORES):
        s = c * BH_PER_CORE
        qp = (qT[s:s + BH_PER_CORE] * QS).reshape(PACKS, 128, L)
        kp = kT[s:s + BH_PER_CORE].reshape(PACKS, 128, L)
        vb = vv[s:s + BH_PER_CORE].reshape(BH_PER_CORE, NT, 128, D)
        v1h = np.zeros((BH_PER_CORE, 128, NT, VW), dtype=np.float32)
        v1h[:, :, :, 0:64] = vb.transpose(0, 2, 1, 3)
        v1h[:, :, :, 64] = 1.0
        # suffix tables (f32, exact V), applied host-side at gather:
        # SUF[t] = sum over s_tiles > t of [V|1|0] rows (col 64 = count)
        vrows = v1h.transpose(0, 2, 1, 3).sum(axis=2)  # [BH, NT, VW]
        suf = np.zeros((BH_PER_CORE, NT, VW), dtype=np.float32)
        suf[:, :-1] = vrows[:, ::-1].cumsum(axis=1)[:, -2::-1]
        sufs.append(suf)
        in_maps.append({
            "qt": np.ascontiguousarray(qp).astype(BF),
            "kt": np.ascontiguousarray(kp).astype(BF),
            "v1": v1h.astype(BF).reshape(BH_PER_CORE, 128, NT * VW),
        })
    return in_maps, sufs


def _gather_outputs(results, sufs):
    outs = []
    for r, suf in zip(results, sufs):
        acc = np.asarray(r["ob"], dtype=np.float32)   # [BH, NJ, 2, 128, 130]
        acc = acc.reshape(BH_PER_CORE, NJ, 2, 128, 2, 65)
        # l_tile t = 4j + 2*lp + i
        acc = acc.transpose(0, 1, 2, 4, 3, 5).reshape(BH_PER_CORE, NT, 128, 65)
        num = acc[..., 0:64] + suf[:, :, None, 0:64]
        den = acc[..., 64] + suf[:, :, None, 64]
        o = num / den[..., None]                      # [BH, NT, 128, 64]
        outs.append(o.reshape(BH_PER_CORE, L, D))
    full = np.concatenate(outs, axis=0)               # [B*H, L, D]
    return np.ascontiguousarray(
        full.reshape(B, H, L, D).transpose(0, 2, 1, 3)
    ).astype(np.float32)  # [B, L, H, D]


def kernel(queries, keys, values, _trace=[False]):
    from concourse.bass_utils import run_bass_kernel_spmd

    queries = np.asarray(queries, dtype=np.float32)
    keys = np.asarray(keys, dtype=np.float32)
    values = np.asarray(values, dtype=np.float32)
    nc = _get_program()
    in_maps, sufs = _shard_inputs(queries, keys, values)
    res = run_bass_kernel_spmd(
        nc, in_maps, core_ids=list(range(NCORES)), trace=_trace[0]
    )
    out = _gather_outputs(res.results, sufs)
    if _trace[0]:
        kernel.last_results = res
    return out


# revision 11
# speedup vs baseline: 1.2276x; 1.0060x over previous
"""FullAttention (non-standard multiplicative causal mask) on 8 TRN2 cores.

Reference (per batch b, head h):
    S = Q @ K^T                      [L, L]
    S = S * tril(ones)               (multiplicative mask: zeros above diag)
    A = softmax(S / sqrt(E))         (masked slots contribute exp(0)=1)
    O = A @ V

Key ideas over the straightforward flash-style kernel:

1. PV layout swap: instead of streaming 512 l-columns with [V|1] stationary
   (cost = l-columns), PV streams only the 65 d-columns with P^T stationary
   (matmul cost is the moving/free size of the output; stationary loads are
   free).  PV column count per head drops 17408 -> 8840.

2. The exp over the causal area (the Act engine's 1 elem/cycle/lane is the
   other bottleneck) is SPLIT between the Scalar/Act engine (exact LUT exp)
   and the Vector/DVE engine running a custom 8-stage DVE op (EXP2BITS_ANT)
   that computes the bf16 BIT PATTERN of 2^w directly:
      r = 128*round(W/128) via the float magic-number trick (exact),
      s = W - r in [-64, 64),
      bits = (a2*s + a1)*s + a0 + r,  written with the f32->int16
      write-convert (RTNE, validated bit-exact on HW), aliased as bf16.
   The required input W = 128*log2(e)*SCALE*(q.k) + 16192 is produced by the
   QK matmul itself: q is pre-scaled host-side and a 65th contraction row
   (bias row q=126.5, k=128) adds the constant -- extra contraction rows are
   free (matmul cost is per-column).  Max rel err of this path ~0.6%, below
   the bf16 input noise floor.  The Act pieces use the same biased W via
   exp(W*ln2/128 - 126.5*ln2).

3. Pieces are assigned to Act vs DVE by a greedy build-time balance of
   modeled busy-time; diagonal pieces (sub-slice APs, 2 free dims) must go
   to Act because the custom op needs a flat 1-free-dim AP (full-tensor
   Src1 + imm2 constraint; [P,1]-broadcast Src1 crashes this firmware).

Sharding: B*H = 32 (b,h) pairs -> 4 per core (2 "packs" of 2 heads).
Per (b,h), chunk-outer loop over l-chunks of 512, s_tile pieces of 128.
The PV output accumulates [l_tile 128, 65] per (head, l_tile) in PSUM
(num cols 0:64, denominator col 64 via the ones column of V1); the host
adds the exact f32 suffix sums (s_tiles > l's tile), divides, reshapes.
"""

import numpy as np

import concourse.bass as bass
import concourse.mybir as mybir
import concourse.tile as tile
from concourse import bacc

F32 = mybir.dt.float32
BF16 = mybir.dt.bfloat16
I16 = mybir.dt.int16
AF = mybir.ActivationFunctionType

B, L, H, E = 2, 2048, 16, 64
D = 64
SCALE = 0.125          # 1/sqrt(64)
NCORES = 8
BH_PER_CORE = (B * H) // NCORES   # 4
PACKS = BH_PER_CORE // 2          # 2
NT = L // 128                     # 16 s-tiles
NJ = L // 512                     # 4 l-chunks
VW = 66                           # [V | 1 | 0pad] (cols 0:65 used)

LN2 = float(np.log(2.0))
QS = 128.0 * SCALE / LN2          # 128*SCALE*log2(e) = 23.0831...
MAGIC = float(np.float32(1.5 * 2 ** 30))
# minimax fit of the (kinked) round-variant bits function on s in [-64, 64):
# bits = p(s) + 128*round(w), s = W - 128*round(w), W = 128*w unbiased
E2A2, E2A1, E2A0 = -0.0024742558182972215, 1.0072715927101399, 16252.395694060908

# cost model (ns) for greedy Act/DVE balance
ACT_NS = 1.0 / 1.2
DVE_NS = 1.0 / 0.96
ACT_INIT = 185.0
DVE_INIT = 125.0

_cached = None


def _register_exp2bits():
    from concourse import dve_ops
    from concourse.dve_spec import Spec, Src0, Src1, C0, C1, C2, lower, _has_src1
    from concourse.dve_uop import DveOpSpec

    name = "EXP2BITS_ANT"
    for op in dve_ops.OPS:
        if op.name == name:
            return op
    m = Src0 + C0
    r = m - C0
    s = Src0 - r
    body = ((s * C1 + C2) * s + Src1) + r

    def ref(in0, in1, s0, s1, imm2):
        in0 = in0.astype(np.float32)
        mm = (in0 + np.float32(s0)).astype(np.float32)
        rr = (mm - np.float32(s0)).astype(np.float32)
        ss = (in0 - rr).astype(np.float32)
        a0 = np.asarray(in1, np.float32)
        if a0.ndim:
            a0 = a0.reshape(in0.shape[0], -1)[:, : in0.shape[-1] if in0.ndim == 2 else 1]
        return ((ss * np.float32(s1) + np.float32(imm2)) * ss + a0 + rr).astype(
            np.float32
        )

    spec = Spec(body=body, reference=ref)
    row = dve_ops._CUSTOM_DVE_ROW_BASE + len(dve_ops.OPS)
    assert row < 0x20
    dve_ops._SUB_OPCODE_FOR_NAME[name] = row
    sha = DveOpSpec(
        name=name, opcode=row, uops=lower(spec, ver="v3"), rd1_en=_has_src1(spec)
    ).sha("v3")
    op = dve_ops.DveOp(name, spec, subdim=False, uops_sha={"v3": sha})
    dve_ops.OPS.append(op)
    dve_ops.CUSTOM_DVE_SPECS[name] = op.spec
    return op


def _build_program():
    EXP2 = _register_exp2bits()
    nc = bacc.Bacc("TRN2", target_bir_lowering=False)
    qt = nc.dram_tensor("qt", [PACKS, 128, L], BF16, kind="ExternalInput")
    kt = nc.dram_tensor("kt", [PACKS, 128, L], BF16, kind="ExternalInput")
    v1d = nc.dram_tensor("v1", [BH_PER_CORE, 128, NT * VW], BF16,
                         kind="ExternalInput")
    ob = nc.dram_tensor("ob", [BH_PER_CORE, NJ, 2, 128, 130], F32,
                        kind="ExternalOutput")

    with tile.TileContext(nc) as tc:
        with (
            tc.tile_pool(name="consts", bufs=1) as consts,
            tc.tile_pool(name="qk_sb", bufs=2) as qk_sb,
            tc.tile_pool(name="v1_sb", bufs=4) as v1_pool,
            tc.tile_pool(name="pt", bufs=8) as pt_pool,
            tc.tile_pool(name="ot_sb", bufs=4) as ot_sb_pool,
            tc.tile_pool(name="qkps", bufs=2, space="PSUM") as qk_ps,
            tc.tile_pool(name="pvps", bufs=4, space="PSUM") as pv_ps,
        ):
            # constants: a0 tile for the custom op (full tensor: [P,1]
            # broadcast Src1 crashes this firmware), Act bias, PE warm tile
            a0full = consts.tile([128, 1024], F32)
            nc.gpsimd.memset(a0full, E2A0)
            warm_sb = consts.tile([128, 64], BF16)
            nc.vector.memset(warm_sb, 0.25)
            # warm the PE p-state during the input-DMA window (ramp needs
            # busy-time, not columns) and pre-load the Act exp table
            warm_ps = qk_ps.tile([128, 2, 512], F32, tag="pp", name="warm")
            for w in range(40):
                nc.tensor.matmul(
                    warm_ps[0:64, 0, 0:64], warm_sb[:, 0:64], warm_sb,
                    start=True, stop=True, skip_group_check=True,
                )
            warm_act = consts.tile([128, 16], BF16)
            nc.scalar.activation(warm_act[:, :], warm_sb[:, 0:16], AF.Exp,
                                 scale=LN2 / 128.0)
            # zero operands for the PV-bank clearing matmul (one K=1 matmul
            # zeroes a whole [128, 260] accumulator region; per-group
            # start=True matmuls corrupt sibling groups in the same bank)
            zlhs = consts.tile([1, 128], BF16)
            nc.vector.memset(zlhs, 0.0)
            zrhs = consts.tile([1, 260], BF16)
            nc.vector.memset(zrhs, 0.0)

            pack_tiles = {}
            busy = {"A": 0.0, "D": 0.0}

            def pick_engine(cols, force_act=False):
                ca = cols * ACT_NS + ACT_INIT
                cd = cols * DVE_NS + DVE_INIT
                if force_act or busy["A"] + ca <= busy["D"] + cd:
                    busy["A"] += ca
                    return "A"
                busy["D"] += cd
                return "D"

            def load_pack(p):
                # chunk-sliced loads: QK of chunk j only needs q cols
                # [512j, 512j+512) and k cols [0, 512(j+1))
                qt_t = qk_sb.tile([128, NJ, 512], BF16, tag="qt", name="qt_t")
                kt_t = qk_sb.tile([128, NJ, 512], BF16, tag="kt", name="kt_t")
                v1l = []
                for hh2 in range(2):
                    v1l.append(v1_pool.tile([128, NJ, 4 * VW], BF16, tag="v1",
                                            name="v1_t"))
                for j in range(NJ):
                    # k on sync, q on gpsimd: first slices land in parallel;
                    # piece 0 only needs k cols [0:128)
                    if j == 0:
                        nc.sync.dma_start(out=kt_t[:, 0, 0:128],
                                          in_=kt[p, :, 0:128])
                        nc.sync.dma_start(out=kt_t[:, 0, 128:512],
                                          in_=kt[p, :, 128:512])
                    else:
                        nc.sync.dma_start(out=kt_t[:, j, :],
                                          in_=kt[p, :, 512 * j:512 * (j + 1)])
                    nc.gpsimd.dma_start(out=qt_t[:, j, :],
                                        in_=qt[p, :, 512 * j:512 * (j + 1)])
                    for hh2 in range(2):
                        nc.sync.dma_start(
                            out=v1l[hh2][:, j, :],
                            in_=v1d[2 * p + hh2, :,
                                    4 * VW * j:4 * VW * (j + 1)],
                        )
                pack_tiles[p] = (qt_t, kt_t, v1l)

            load_pack(0)
            for pack in range(PACKS):
                qt_t, kt_t, v1 = pack_tiles.pop(pack)

                j_iter = range(NJ) if pack == 0 else range(NJ - 1, -1, -1)
                for j in j_iter:
                    if pack == 0 and j == 2 and pack + 1 < PACKS:
                        load_pack(pack + 1)
                    nk = 4 * j + 4          # s_tiles participating causally
                    # PV accumulators: 2 PSUM tiles per chunk, one per
                    # l_tile pair (hh, li&1, col); zeroed by a single K=1
                    # matmul, then all PV matmuls accumulate (start=False)
                    pv = [pv_ps.tile([128, 2, 2, 65], F32, tag="pv",
                                     name="pv") for _ in range(2)]
                    for lp in range(2):
                        nc.tensor.matmul(
                            pv[lp].rearrange("p a b c -> p (a b c)")[:, :],
                            zlhs[:, :], zrhs[:, :],
                            start=True, stop=False, skip_group_check=True,
                        )

                    pending_pv = []   # list of per-piece lists of closures

                    def emit_pv(hh, k, li, pt_t, j=j, pv=pv, v1=v1):
                        nc.tensor.matmul(
                            pv[li // 2][:, hh, li % 2, :],
                            pt_t[:, hh, 128 * li:128 * li + 128],
                            v1[hh][:, k // 4,
                                   VW * (k % 4):VW * (k % 4) + 65],
                            start=False,
                            stop=(k == 4 * j + li),
                            skip_group_check=True,
                        )

                    def exp_piece(eng, pt_ap, pp_ap, ncols):
                        if eng == "A":
                            nc.scalar.activation(
                                pt_ap, pp_ap,
                                AF.Exp, scale=LN2 / 128.0,
                            )
                        else:
                            nc.vector._custom_dve(
                                EXP2,
                                out=pt_ap.bitcast(I16),
                                in0=pp_ap,
                                in1=a0full[:, 0:ncols],
                                s0=MAGIC,
                                s1=E2A2,
                                imm2=E2A1,
                            )

                    # pieces processed in PAIRS: all 4 QK matmuls of two
                    # pieces back-to-back (the PE pays a ~0.4us restart when
                    # a QK follows a PV matmul; pairing halves transitions),
                    # then exp(k) and exp(k+1) launch on OPPOSITE engines
                    kk_ = 0
                    while kk_ < nk:
                        pair = [kk_] if kk_ + 1 >= nk else [kk_, kk_ + 1]
                        kk_ += len(pair)
                        # drain delayed PV BEFORE the QKs: the PE is
                        # in-order, so ready PV work must sit ahead of a QK
                        # that may stall on its PSUM bank
                        depth = 2 if (pack == PACKS - 1 and j == 0) else 6
                        while len(pending_pv) > depth:
                            for fn in pending_pv.pop(0):
                                fn()
                        pieces = []
                        for k in pair:
                            pp = qk_ps.tile([128, 2, 512], F32, tag="pp",
                                            name="pp")
                            pt_t = pt_pool.tile([128, 2, 512], BF16,
                                                tag="pt", name="pt")
                            m = k - 4 * j
                            qoff = 128 * max(0, m)
                            for hh in range(2):
                                r0 = 64 * hh
                                nc.tensor.matmul(
                                    pp[:, hh, qoff:512],
                                    kt_t[r0:r0 + 64, k // 4,
                                         128 * (k % 4):128 * (k % 4) + 128],
                                    qt_t[r0:r0 + 64, j, qoff:512],
                                    start=True, stop=True,
                                )
                            pieces.append((k, m, qoff, pp, pt_t))
                        # exp: within a pair, strict engine alternation
                        # (first by greedy, second the opposite); diagonal
                        # pieces split per-head across both engines
                        for idx, (k, m, qoff, pp, pt_t) in enumerate(pieces):
                            cols1 = 512 - qoff
                            if m >= 1:
                                e0 = pick_engine(cols1)
                                e1 = "D" if e0 == "A" else "A"
                                busy[e1] += (cols1 * (ACT_NS if e1 == "A"
                                                      else DVE_NS)
                                             + (ACT_INIT if e1 == "A"
                                                else DVE_INIT))
                                for hh, eng in ((0, e0), (1, e1)):
                                    exp_piece(eng,
                                              pt_t[:, hh, qoff:512],
                                              pp[:, hh, qoff:512], cols1)
                            elif idx == 0:
                                eng0 = pick_engine(2 * cols1)
                                if eng0 == "A":
                                    exp_piece("A", pt_t[:, :, :],
                                              pp[:, :, :], 1024)
                                else:
                                    exp_piece(
                                        "D",
                                        pt_t.rearrange(
                                            "p a b -> p (a b)")[:, :],
                                        pp.rearrange(
                                            "p a b -> p (a b)")[:, :],
                                        1024)
                            else:
                                eng1 = "D" if eng0 == "A" else "A"
                                busy[eng1] += (2 * cols1 *
                                               (ACT_NS if eng1 == "A"
                                                else DVE_NS)
                                               + (ACT_INIT if eng1 == "A"
                                                  else DVE_INIT))
                                if eng1 == "A":
                                    exp_piece("A", pt_t[:, :, :],
                                              pp[:, :, :], 1024)
                                else:
                                    exp_piece(
                                        "D",
                                        pt_t.rearrange(
                                            "p a b -> p (a b)")[:, :],
                                        pp.rearrange(
                                            "p a b -> p (a b)")[:, :],
                                        1024)
                        for k, m, qoff, pp, pt_t in pieces:
                            piece_pv = []
                            for hh in range(2):
                                if m >= 0:         # diagonal piece fixup
                                    nc.gpsimd.affine_select(
                                        out=pt_t[:, hh,
                                                 128 * m:128 * m + 128],
                                        in_=pt_t[:, hh,
                                                 128 * m:128 * m + 128],
                                        compare_op=mybir.AluOpType.is_ge,
                                        fill=1.0,
                                        base=0,
                                        pattern=[[1, 128]],
                                        channel_multiplier=-1,
                                    )
                                for li in range(max(0, m), 4):
                                    piece_pv.append(
                                        lambda hh=hh, k=k, li=li, p=pt_t:
                                        emit_pv(hh, k, li, p)
                                    )
                            pending_pv.append(piece_pv)
                    for plist in pending_pv:
                        for fn in plist:
                            fn()
                    pending_pv = []

                    # ship [128, 2, 2, 65] f32 per l_tile-pair; suffix add +
                    # divide happen on the host
                    for lp in range(2):
                        ot_s = ot_sb_pool.tile([128, 2, 2, 65], F32,
                                               tag="ots")
                        ccols = 260.0
                        ca = ccols * ACT_NS + ACT_INIT
                        cd = ccols * DVE_NS + DVE_INIT
                        if busy["A"] + ca <= busy["D"] + cd:
                            busy["A"] += ca
                            nc.scalar.copy(ot_s[:, :, :, :], pv[lp][:, :, :, :])
                        else:
                            busy["D"] += cd
                            nc.vector.tensor_copy(ot_s[:, :, :, :], pv[lp][:, :, :, :])
                        for hh in range(2):
                            bh = 2 * pack + hh
                            nc.sync.dma_start(
                                out=ob[bh, j, lp],
                                in_=ot_s.rearrange("p a b c -> p a (b c)")[
                                    :, hh, :],
                            )

    nc.compile()
    return nc


def _get_program():
    global _cached
    if _cached is None:
        _cached = _build_program()
    return _cached


def _shard_inputs(queries, keys, values):
    import ml_dtypes
    BF = ml_dtypes.bfloat16
    # [B, L, H, E] -> [B, H, E, L] -> [BH, E, L]
    qT = np.ascontiguousarray(queries.transpose(0, 2, 3, 1)).reshape(B * H, E, L)
    kT = np.ascontiguousarray(keys.transpose(0, 2, 3, 1)).reshape(B * H, E, L)
    # [B, L, H, D] -> [BH, L, D]
    vv = np.ascontiguousarray(values.transpose(0, 2, 1, 3)).reshape(B * H, L, D)
    in_maps = []
    sufs = []
    for c in range(NCORES):
        s = c * BH_PER_CORE
        qp = (qT[s:s + BH_PER_CORE] * QS).reshape(PACKS, 128, L)
        kp = kT[s:s + BH_PER_CORE].reshape(PACKS, 128, L)
        vb = vv[s:s + BH_PER_CORE].reshape(BH_PER_CORE, NT, 128, D)
        v1h = np.zeros((BH_PER_CORE, 128, NT, VW), dtype=np.float32)
        v1h[:, :, :, 0:64] = vb.transpose(0, 2, 1, 3)
        v1h[:, :, :, 64] = 1.0
        # suffix tables (f32, exact V), applied host-side at gather:
        # SUF[t] = sum over s_tiles > t of [V|1|0] rows (col 64 = count)
        vrows = v1h.transpose(0, 2, 1, 3).sum(axis=2)  # [BH, NT, VW]
        suf = np.zeros((BH_PER_CORE, NT, VW), dtype=np.float32)
        suf[:, :-1] = vrows[:, ::-1].cumsum(axis=1)[:, -2::-1]
        sufs.append(suf)
        in_maps.append({
            "qt": np.ascontiguousarray(qp).astype(BF),
            "kt": np.ascontiguousarray(kp).astype(BF),
            "v1": v1h.astype(BF).reshape(BH_PER_CORE, 128, NT * VW),
        })
    return in_maps, sufs


def _gather_outputs(results, sufs):
    outs = []
    for r, suf in zip(results, sufs):
        acc = np.asarray(r["ob"], dtype=np.float32)   # [BH, NJ, 2, 128, 130]
        acc = acc.reshape(BH_PER_CORE, NJ, 2, 128, 2, 65)
        # l_tile t = 4j + 2*lp + i
        acc = acc.transpose(0, 1, 2, 4, 3, 5).reshape(BH_PER_CORE, NT, 128, 65)
        num = acc[..., 0:64] + suf[:, :, None, 0:64]
        den = acc[..., 64] + suf[:, :, None, 64]
        o = num / den[..., None]                      # [BH, NT, 128, 64]
        outs.append(o.reshape(BH_PER_CORE, L, D))
    full = np.concatenate(outs, axis=0)               # [B*H, L, D]
    return np.ascontiguousarray(
        full.reshape(B, H, L, D).transpose(0, 2, 1, 3)
    ).astype(np.float32)  # [B, L, H, D]


def kernel(queries, keys, values, _trace=[False]):
    from concourse.bass_utils import run_bass_kernel_spmd

    queries = np.asarray(queries, dtype=np.float32)
    keys = np.asarray(keys, dtype=np.float32)
    values = np.asarray(values, dtype=np.float32)
    nc = _get_program()
    in_maps, sufs = _shard_inputs(queries, keys, values)
    res = run_bass_kernel_spmd(
        nc, in_maps, core_ids=list(range(NCORES)), trace=_trace[0]
    )
    out = _gather_outputs(res.results, sufs)
    if _trace[0]:
        kernel.last_results = res
    return out


# revision 12
# speedup vs baseline: 1.5315x; 1.2475x over previous
"""FullAttention (non-standard multiplicative causal mask) on 8 TRN2 cores.

Reference (per batch b, head h):
    S = Q @ K^T                      [L, L]
    S = S * tril(ones)               (multiplicative mask: zeros above diag)
    A = softmax(S / sqrt(E))         (masked slots contribute exp(0)=1)
    O = A @ V

Key ideas over the straightforward flash-style kernel:

1. PV layout swap: instead of streaming 512 l-columns with [V|1] stationary
   (cost = l-columns), PV streams only the 65 d-columns with P^T stationary
   (matmul cost is the moving/free size of the output; stationary loads are
   free).  PV column count per head drops 17408 -> 8840.

2. The exp over the causal area (the Act engine's 1 elem/cycle/lane is the
   other bottleneck) is SPLIT between the Scalar/Act engine (exact LUT exp)
   and the Vector/DVE engine running a custom 8-stage DVE op (EXP2BITS_ANT)
   that computes the bf16 BIT PATTERN of 2^w directly:
      r = 128*round(W/128) via the float magic-number trick (exact),
      s = W - r in [-64, 64),
      bits = (a2*s + a1)*s + a0 + r,  written with the f32->int16
      write-convert (RTNE, validated bit-exact on HW), aliased as bf16.
   The required input W = 128*log2(e)*SCALE*(q.k) + 16192 is produced by the
   QK matmul itself: q is pre-scaled host-side and a 65th contraction row
   (bias row q=126.5, k=128) adds the constant -- extra contraction rows are
   free (matmul cost is per-column).  Max rel err of this path ~0.6%, below
   the bf16 input noise floor.  The Act pieces use the same biased W via
   exp(W*ln2/128 - 126.5*ln2).

3. Pieces are assigned to Act vs DVE by a greedy build-time balance of
   modeled busy-time; diagonal pieces (sub-slice APs, 2 free dims) must go
   to Act because the custom op needs a flat 1-free-dim AP (full-tensor
   Src1 + imm2 constraint; [P,1]-broadcast Src1 crashes this firmware).

Sharding: B*H = 32 (b,h) pairs -> 4 per core (2 "packs" of 2 heads).
Per (b,h), chunk-outer loop over l-chunks of 512, s_tile pieces of 128.
The PV output accumulates [l_tile 128, 65] per (head, l_tile) in PSUM
(num cols 0:64, denominator col 64 via the ones column of V1); the host
adds the exact f32 suffix sums (s_tiles > l's tile), divides, reshapes.
"""

import numpy as np

import concourse.bass as bass
import concourse.mybir as mybir
import concourse.tile as tile
from concourse import bacc

F32 = mybir.dt.float32
BF16 = mybir.dt.bfloat16
I16 = mybir.dt.int16
AF = mybir.ActivationFunctionType

B, L, H, E = 2, 2048, 16, 64
D = 64
SCALE = 0.125          # 1/sqrt(64)
NCORES = 8
BH_PER_CORE = (B * H) // NCORES   # 4
PACKS = BH_PER_CORE // 2          # 2
NT = L // 128                     # 16 s-tiles
NJ = L // 512                     # 4 l-chunks
VW = 66                           # [V | 1 | 0pad] (cols 0:65 used)

LN2 = float(np.log(2.0))
QS = 128.0 * SCALE / LN2          # 128*SCALE*log2(e) = 23.0831...
MAGIC = float(np.float32(1.5 * 2 ** 30))
# minimax fit of the (kinked) round-variant bits function on s in [-64, 64):
# bits = p(s) + 128*round(w), s = W - 128*round(w), W = 128*w unbiased
E2A2, E2A1, E2A0 = -0.0024742558182972215, 1.0072715927101399, 16252.395694060908

# cost model (ns) for greedy Act/DVE balance
ACT_NS = 1.0 / 1.2
DVE_NS = 1.0 / 0.96
ACT_INIT = 185.0
DVE_INIT = 125.0

_cached = None


def _register_exp2bits():
    from concourse import dve_ops
    from concourse.dve_spec import Spec, Src0, Src1, C0, C1, C2, lower, _has_src1
    from concourse.dve_uop import DveOpSpec

    name = "EXP2BITS_ANT"
    for op in dve_ops.OPS:
        if op.name == name:
            return op
    m = Src0 + C0
    r = m - C0
    s = Src0 - r
    body = ((s * C1 + C2) * s + Src1) + r

    def ref(in0, in1, s0, s1, imm2):
        in0 = in0.astype(np.float32)
        mm = (in0 + np.float32(s0)).astype(np.float32)
        rr = (mm - np.float32(s0)).astype(np.float32)
        ss = (in0 - rr).astype(np.float32)
        a0 = np.asarray(in1, np.float32)
        if a0.ndim:
            a0 = a0.reshape(in0.shape[0], -1)[:, : in0.shape[-1] if in0.ndim == 2 else 1]
        return ((ss * np.float32(s1) + np.float32(imm2)) * ss + a0 + rr).astype(
            np.float32
        )

    spec = Spec(body=body, reference=ref)
    row = dve_ops._CUSTOM_DVE_ROW_BASE + len(dve_ops.OPS)
    assert row < 0x20
    dve_ops._SUB_OPCODE_FOR_NAME[name] = row
    sha = DveOpSpec(
        name=name, opcode=row, uops=lower(spec, ver="v3"), rd1_en=_has_src1(spec)
    ).sha("v3")
    op = dve_ops.DveOp(name, spec, subdim=False, uops_sha={"v3": sha})
    dve_ops.OPS.append(op)
    dve_ops.CUSTOM_DVE_SPECS[name] = op.spec
    return op


def _build_program():
    EXP2 = _register_exp2bits()
    nc = bacc.Bacc("TRN2", target_bir_lowering=False)
    qt = nc.dram_tensor("qt", [PACKS, 128, L], BF16, kind="ExternalInput")
    kt = nc.dram_tensor("kt", [PACKS, 128, L], BF16, kind="ExternalInput")
    v1d = nc.dram_tensor("v1", [BH_PER_CORE, 128, NT * VW], BF16,
                         kind="ExternalInput")
    ob = nc.dram_tensor("ob", [BH_PER_CORE, NJ, 2, 128, 130], F32,
                        kind="ExternalOutput")

    with tile.TileContext(nc) as tc:
        with (
            tc.tile_pool(name="consts", bufs=1) as consts,
            tc.tile_pool(name="qk_sb", bufs=2) as qk_sb,
            tc.tile_pool(name="v1_sb", bufs=4) as v1_pool,
            tc.tile_pool(name="pt", bufs=8) as pt_pool,
            tc.tile_pool(name="ot_sb", bufs=4) as ot_sb_pool,
            tc.tile_pool(name="qkps", bufs=3, space="PSUM") as qk_ps,
            tc.tile_pool(name="pvps", bufs=2, space="PSUM") as pv_ps,
        ):
            # constants: a0 tile for the custom op (full tensor: [P,1]
            # broadcast Src1 crashes this firmware), Act bias, PE warm tile
            a0full = consts.tile([128, 1024], F32)
            nc.gpsimd.memset(a0full, E2A0)
            warm_sb = consts.tile([128, 64], BF16)
            nc.vector.memset(warm_sb, 0.25)
            # warm the PE p-state during the input-DMA window (ramp needs
            # busy-time, not columns) and pre-load the Act exp table
            warm_ps = qk_ps.tile([128, 2, 512], F32, tag="pp", name="warm")
            for w in range(40):
                nc.tensor.matmul(
                    warm_ps[0:64, 0, 0:64], warm_sb[:, 0:64], warm_sb,
                    start=True, stop=True, skip_group_check=True,
                )
            warm_act = consts.tile([128, 16], BF16)
            nc.scalar.activation(warm_act[:, :], warm_sb[:, 0:16], AF.Exp,
                                 scale=LN2 / 128.0)
            # zero operands for the PV-bank clearing matmul (one K=1 matmul
            # zeroes a whole [128, 260] accumulator region; per-group
            # start=True matmuls corrupt sibling groups in the same bank)
            zlhs = consts.tile([1, 128], BF16)
            nc.vector.memset(zlhs, 0.0)
            zrhs = consts.tile([1, 260], BF16)
            nc.vector.memset(zrhs, 0.0)

            pack_tiles = {}
            busy = {"A": 0.0, "D": 0.0}

            def pick_engine(cols, force_act=False):
                ca = cols * ACT_NS + ACT_INIT
                cd = cols * DVE_NS + DVE_INIT
                if force_act or busy["A"] + ca <= busy["D"] + cd:
                    busy["A"] += ca
                    return "A"
                busy["D"] += cd
                return "D"

            def load_pack(p):
                # chunk-sliced loads: QK of chunk j only needs q cols
                # [512j, 512j+512) and k cols [0, 512(j+1))
                qt_t = qk_sb.tile([128, NJ, 512], BF16, tag="qt", name="qt_t")
                kt_t = qk_sb.tile([128, NJ, 512], BF16, tag="kt", name="kt_t")
                v1l = []
                for hh2 in range(2):
                    v1l.append(v1_pool.tile([128, NJ, 4 * VW], BF16, tag="v1",
                                            name="v1_t"))
                for j in range(NJ):
                    # k on sync, q on gpsimd: first slices land in parallel;
                    # piece 0 only needs k cols [0:128)
                    if j == 0:
                        nc.sync.dma_start(out=kt_t[:, 0, 0:128],
                                          in_=kt[p, :, 0:128])
                        nc.sync.dma_start(out=kt_t[:, 0, 128:512],
                                          in_=kt[p, :, 128:512])
                    else:
                        nc.sync.dma_start(out=kt_t[:, j, :],
                                          in_=kt[p, :, 512 * j:512 * (j + 1)])
                    nc.gpsimd.dma_start(out=qt_t[:, j, :],
                                        in_=qt[p, :, 512 * j:512 * (j + 1)])
                    for hh2 in range(2):
                        nc.sync.dma_start(
                            out=v1l[hh2][:, j, :],
                            in_=v1d[2 * p + hh2, :,
                                    4 * VW * j:4 * VW * (j + 1)],
                        )
                pack_tiles[p] = (qt_t, kt_t, v1l)

            load_pack(0)
            for pack in range(PACKS):
                qt_t, kt_t, v1 = pack_tiles.pop(pack)

                j_iter = range(NJ) if pack == 0 else range(NJ - 1, -1, -1)
                for j in j_iter:
                    if pack == 0 and j == 2 and pack + 1 < PACKS:
                        load_pack(pack + 1)
                    nk = 4 * j + 4          # s_tiles participating causally
                    # PV accumulators: 2 PSUM tiles per chunk, one per
                    # l_tile pair (hh, li&1, col); zeroed by a single K=1
                    # matmul, then all PV matmuls accumulate (start=False)
                    pv = [pv_ps.tile([128, 2, 2, 65], F32, tag="pv",
                                     name="pv") for _ in range(2)]

                    pending_pv = []   # list of per-piece lists of closures

                    def emit_pv(hh, k, li, pt_t, j=j, pv=pv, v1=v1):
                        nc.tensor.matmul(
                            pv[li // 2][:, hh, li % 2, :],
                            pt_t[:, hh, 128 * li:128 * li + 128],
                            v1[hh][:, k // 4,
                                   VW * (k % 4):VW * (k % 4) + 65],
                            start=False,
                            stop=(k == 4 * j + li),
                            skip_group_check=True,
                        )

                    def exp_piece(eng, pt_ap, pp_ap, ncols):
                        if eng == "A":
                            nc.scalar.activation(
                                pt_ap, pp_ap,
                                AF.Exp, scale=LN2 / 128.0,
                            )
                        else:
                            nc.vector._custom_dve(
                                EXP2,
                                out=pt_ap.bitcast(I16),
                                in0=pp_ap,
                                in1=a0full[:, 0:ncols],
                                s0=MAGIC,
                                s1=E2A2,
                                imm2=E2A1,
                            )

                    for k in range(nk):             # s_tile pieces
                        # drain delayed PV BEFORE the QK: the PE is
                        # in-order, so ready PV work must sit ahead of a QK
                        # that may stall on its PSUM bank
                        depth = 2 if (pack == PACKS - 1 and j == 0) else 6
                        while len(pending_pv) > depth:
                            for fn in pending_pv.pop(0):
                                fn()
                        pp = qk_ps.tile([128, 2, 512], F32, tag="pp",
                                        name="pp")
                        pt_t = pt_pool.tile([128, 2, 512], BF16, tag="pt",
                                            name="pt")
                        m = k - 4 * j
                        qoff = 128 * max(0, m)
                        for hh in range(2):
                            r0 = 64 * hh
                            nc.tensor.matmul(
                                pp[:, hh, qoff:512],
                                kt_t[r0:r0 + 64, k // 4,
                                     128 * (k % 4):128 * (k % 4) + 128],
                                qt_t[r0:r0 + 64, j, qoff:512],
                                start=True, stop=True,
                            )
                        if k == 0:
                            # zero the chunk's PV accumulator banks (one K=1
                            # matmul per bank; per-group start=True matmuls
                            # corrupt sibling groups).  Deferred past the
                            # first QK so the WAR wait on the previous
                            # chunk's drain copy never stalls the PE.
                            for lp in range(2):
                                nc.tensor.matmul(
                                    pv[lp].rearrange(
                                        "p a b c -> p (a b c)")[:, :],
                                    zlhs[:, :], zrhs[:, :],
                                    start=True, stop=False,
                                    skip_group_check=True,
                                )
                        # exp: diagonal pieces split per-head across BOTH
                        # engines; full pieces merged 2-head on the
                        # greedy-min engine (alternates when balanced)
                        cols1 = 512 - qoff
                        if m >= 1:
                            e0 = pick_engine(cols1)
                            e1 = "D" if e0 == "A" else "A"
                            busy[e1] += (cols1 * (ACT_NS if e1 == "A"
                                                  else DVE_NS)
                                         + (ACT_INIT if e1 == "A"
                                            else DVE_INIT))
                            for hh, eng in ((0, e0), (1, e1)):
                                exp_piece(eng,
                                          pt_t[:, hh, qoff:512],
                                          pp[:, hh, qoff:512], cols1)
                        else:
                            eng = pick_engine(2 * cols1)
                            if eng == "A":
                                exp_piece("A", pt_t[:, :, :], pp[:, :, :],
                                          1024)
                            else:
                                exp_piece(
                                    "D",
                                    pt_t.rearrange("p a b -> p (a b)")[:, :],
                                    pp.rearrange("p a b -> p (a b)")[:, :],
                                    1024)
                        piece_pv = []
                        for hh in range(2):
                            if m >= 0:             # diagonal piece fixup
                                nc.gpsimd.affine_select(
                                    out=pt_t[:, hh, 128 * m:128 * m + 128],
                                    in_=pt_t[:, hh, 128 * m:128 * m + 128],
                                    compare_op=mybir.AluOpType.is_ge,
                                    fill=1.0,
                                    base=0,
                                    pattern=[[1, 128]],
                                    channel_multiplier=-1,
                                )
                            for li in range(max(0, m), 4):
                                piece_pv.append(
                                    lambda hh=hh, k=k, li=li, p=pt_t:
                                    emit_pv(hh, k, li, p)
                                )
                        pending_pv.append(piece_pv)
                    for plist in pending_pv:
                        for fn in plist:
                            fn()
                    pending_pv = []

                    # ship [128, 2, 2, 65] f32 per l_tile-pair; suffix add +
                    # divide happen on the host
                    for lp in range(2):
                        ot_s = ot_sb_pool.tile([128, 2, 2, 65], F32,
                                               tag="ots")
                        ccols = 260.0
                        ca = ccols * ACT_NS + ACT_INIT
                        cd = ccols * DVE_NS + DVE_INIT
                        if busy["A"] + ca <= busy["D"] + cd:
                            busy["A"] += ca
                            nc.scalar.copy(ot_s[:, :, :, :], pv[lp][:, :, :, :])
                        else:
                            busy["D"] += cd
                            nc.vector.tensor_copy(ot_s[:, :, :, :], pv[lp][:, :, :, :])
                        for hh in range(2):
                            bh = 2 * pack + hh
                            nc.sync.dma_start(
                                out=ob[bh, j, lp],
                                in_=ot_s.rearrange("p a b c -> p a (b c)")[
                                    :, hh, :],
                            )

    nc.compile()
    return nc


def _get_program():
    global _cached
    if _cached is None:
        _cached = _build_program()
    return _cached


def _shard_inputs(queries, keys, values):
    import ml_dtypes
    BF = ml_dtypes.bfloat16
    # [B, L, H, E] -> [B, H, E, L] -> [BH, E, L]
    qT = np.ascontiguousarray(queries.transpose(0, 2, 3, 1)).reshape(B * H, E, L)
    kT = np.ascontiguousarray(keys.transpose(0, 2, 3, 1)).reshape(B * H, E, L)
    # [B, L, H, D] -> [BH, L, D]
    vv = np.ascontiguousarray(values.transpose(0, 2, 1, 3)).reshape(B * H, L, D)
    in_maps = []
    sufs = []
    for c in range(NCORES):
        s = c * BH_PER_CORE
        qp = (qT[s:s + BH_PER_CORE] * QS).reshape(PACKS, 128, L)
        kp = kT[s:s + BH_PER_CORE].reshape(PACKS, 128, L)
        vb = vv[s:s + BH_PER_CORE].reshape(BH_PER_CORE, NT, 128, D)
        v1h = np.zeros((BH_PER_CORE, 128, NT, VW), dtype=np.float32)
        v1h[:, :, :, 0:64] = vb.transpose(0, 2, 1, 3)
        v1h[:, :, :, 64] = 1.0
        # suffix tables (f32, exact V), applied host-side at gather:
        # SUF[t] = sum over s_tiles > t of [V|1|0] rows (col 64 = count)
        vrows = v1h.transpose(0, 2, 1, 3).sum(axis=2)  # [BH, NT, VW]
        suf = np.zeros((BH_PER_CORE, NT, VW), dtype=np.float32)
        suf[:, :-1] = vrows[:, ::-1].cumsum(axis=1)[:, -2::-1]
        sufs.append(suf)
        in_maps.append({
            "qt": np.ascontiguousarray(qp).astype(BF),
            "kt": np.ascontiguousarray(kp).astype(BF),
            "v1": v1h.astype(BF).reshape(BH_PER_CORE, 128, NT * VW),
        })
    return in_maps, sufs


def _gather_outputs(results, sufs):
    outs = []
    for r, suf in zip(results, sufs):
        acc = np.asarray(r["ob"], dtype=np.float32)   # [BH, NJ, 2, 128, 130]
        acc = acc.reshape(BH_PER_CORE, NJ, 2, 128, 2, 65)
        # l_tile t = 4j + 2*lp + i
        acc = acc.transpose(0, 1, 2, 4, 3, 5).reshape(BH_PER_CORE, NT, 128, 65)
        num = acc[..., 0:64] + suf[:, :, None, 0:64]
        den = acc[..., 64] + suf[:, :, None, 64]
        o = num / den[..., None]                      # [BH, NT, 128, 64]
        outs.append(o.reshape(BH_PER_CORE, L, D))
    full = np.concatenate(outs, axis=0)               # [B*H, L, D]
    return np.ascontiguousarray(
        full.reshape(B, H, L, D).transpose(0, 2, 1, 3)
    ).astype(np.float32)  # [B, L, H, D]


def kernel(queries, keys, values, _trace=[False]):
    from concourse.bass_utils import run_bass_kernel_spmd

    queries = np.asarray(queries, dtype=np.float32)
    keys = np.asarray(keys, dtype=np.float32)
    values = np.asarray(values, dtype=np.float32)
    nc = _get_program()
    in_maps, sufs = _shard_inputs(queries, keys, values)
    res = run_bass_kernel_spmd(
        nc, in_maps, core_ids=list(range(NCORES)), trace=_trace[0]
    )
    out = _gather_outputs(res.results, sufs)
    if _trace[0]:
        kernel.last_results = res
    return out
